# revision 12
# baseline (speedup 1.0000x reference)
"""Trainium2 Bass kernel for the LSTM decoder (embed -> LSTM -> vocab linear -> pack).

Strategy: pure data-parallel over batch (32 rows/core on 8 cores), zero
collectives.  All matmuls in bf16 with fp32 PSUM accumulation:
  phase 1: input-side gates Gx for all packed (t,b) columns in one blocked
           matmul (moving dim ~= packed columns), bias folded in via ACT.
  phase 2: 20 sequential LSTM steps; gates in transposed layout
           [4H partition-tiles, batch cols]; h kept bf16 as the stationary
           matmul operand; elementwise on DVE/ACT.
  phase 3: vocab-blocked output linear over packed hs columns, streaming
           lin_W tiles from HBM; outputs transposed [V, packed] per core.

Host side: embedding gather, per-core transposed/packed layout prep, and
final unpack/transpose back to the reference's packed row order.
"""

import sys, json, os

sys.path.insert(0, '/opt/trn_rl_repo')

import numpy as np
import ml_dtypes

import concourse.bass as bass
import concourse.tile as tile
from concourse import mybir
from concourse.bass_utils import run_bass_kernel_spmd

bf16 = ml_dtypes.bfloat16

V, E, H, B, T = 10000, 512, 1024, 256, 20
NC = 8           # cores
BS = B // NC     # batch rows per core
KE = E // 128    # 4 contraction chunks over E
KH = H // 128    # 8 contraction chunks over H
GT = 4 * H // 128  # 32 gate tiles of 128 rows
VT = (V + 127) // 128  # 79 vocab tiles
VP = VT * 128    # 10112 padded vocab

# gate m-tile bank order: bank A = [i slices 0..7, g slices 0..7],
# bank B = [f slices 0..7, o slices 0..7].  rowbase(mi) = row offset in 4H.
def _rowbase(mi):
    if mi < 8:      # i
        return 0 * H + 128 * mi
    elif mi < 16:   # g
        return 2 * H + 128 * (mi - 8)
    elif mi < 24:   # f
        return 1 * H + 128 * (mi - 16)
    else:           # o
        return 3 * H + 128 * (mi - 24)


# ---------------------------------------------------------------------------
# Workarounds for this container's walrus build (max ONE sync wait per
# instruction): split multi-wait instructions into preceding single-wait
# EventSemaphore ops on the same engine, and emit the Tile end-of-kernel
# drain as individual waits.
# ---------------------------------------------------------------------------

def _split_multiwaits(j):
    ctr = 0
    for fn in j['functions']:
        for bb in fn['blocks']:
            new_insts = []
            for inst in bb['instructions']:
                si = inst.get('sync_info')
                if si:
                    waits = si.get('on_wait') or []
                    if len(waits) > 1:
                        for w in waits[:-1]:
                            ctr += 1
                            nop = {
                                "engine": inst["engine"], "ins": [], "outs": [],
                                "name": f"wsplit-{ctr}",
                                "opcode": "EventSemaphore",
                                "sync_info": {"on_update": [], "on_wait": [w]},
                            }
                            if "debug" in inst:
                                nop["debug"] = inst["debug"]
                            new_insts.append(nop)
                        si['on_wait'] = [waits[-1]]
                new_insts.append(inst)
            bb['instructions'] = new_insts
    return j


class PatchedBass(bass.Bass):
    def to_json_bytes(self):
        j = json.loads(super().to_json_bytes())
        return json.dumps(_split_multiwaits(j)).encode()


class SplitDrainTC(tile.TileContext):
    def _drain_and_barrier(self, tick_clock, wait_clock):
        gc = tick_clock.global_clock
        for proc_idx, sem in sorted(wait_clock.sems.allocated().items()):
            tick = gc[proc_idx]
            if tick > 0:
                # DMA queue sems are bumped by 16 per transfer; the vector
                # clock ticks by 1 per transfer.
                name = getattr(sem, 'name', '') or ''
                mult = 16 if name.startswith('DMA') else 1
                self.nc.sync.wait_ge(sem, tick * mult)
        self.nc.sync.drain()
        self.nc.all_engine_barrier()
        assert self.sems is not None
        popped = self.nc._tile_sem_poison_stack.pop()
        assert popped is self._sem_poison
        self.nc.clear_and_free_semaphores(list(self.sems.allocated().values()))
        self.nc.all_engine_barrier()


# ---------------------------------------------------------------------------
# Device program
# ---------------------------------------------------------------------------

def _col_chunks(n, maxc=512):
    """Split [0, n) into contiguous chunks of size <= maxc."""
    out = []
    s = 0
    while s < n:
        c = min(maxc, n - s)
        out.append((s, c))
        s += c
    return out


def build_program(n_sched, pack, loop_n=1, dyn_loop=False):
    """n_sched: per-step uniform column counts (len <= T, all >= 1).
    pack = sum(n_sched).  When dyn_loop, the whole pipeline is wrapped in a
    runtime-count For_i loop (count from input tensor "nrep") for timing."""
    f32 = mybir.dt.float32
    b16 = mybir.dt.bfloat16
    steps = len(n_sched)
    off = np.concatenate([[0], np.cumsum(n_sched)]).astype(int)

    nc = PatchedBass("TRN2", target_bir_lowering=False, debug=False,
                     num_devices=NC)

    if dyn_loop:
        nrep = nc.dram_tensor("nrep", [1, 1], mybir.dt.int32, kind="ExternalInput")
    xsT = nc.dram_tensor("xsT", [128, KE * pack], b16, kind="ExternalInput")
    wihT = nc.dram_tensor("wihT", [128, KE * 4 * H], b16, kind="ExternalInput")
    whhT = nc.dram_tensor("whhT", [128, KH * 4 * H], b16, kind="ExternalInput")
    linT = nc.dram_tensor("linT", [128, VT * KH * 128], b16, kind="ExternalInput")
    biasv = nc.dram_tensor("biasv", [128, GT], f32, kind="ExternalInput")
    h0T = nc.dram_tensor("h0T", [128, KH * BS], b16, kind="ExternalInput")
    c0T = nc.dram_tensor("c0T", [128, KH * BS], f32, kind="ExternalInput")
    outT = nc.dram_tensor("outT", [VP, pack], f32, kind="ExternalOutput")

    with SplitDrainTC(nc) as tc:
        from contextlib import ExitStack
        with ExitStack() as ctx:
            consts = ctx.enter_context(tc.tile_pool(name="consts", bufs=1))
            gxp = ctx.enter_context(tc.tile_pool(name="gxsb", bufs=1))
            hsp = ctx.enter_context(tc.tile_pool(name="hssb", bufs=1))
            hp = ctx.enter_context(tc.tile_pool(name="hbuf", bufs=2))
            cp = ctx.enter_context(tc.tile_pool(name="cbuf", bufs=2))
            tmp = ctx.enter_context(tc.tile_pool(name="tmp", bufs=2))
            linw = ctx.enter_context(tc.tile_pool(name="linw", bufs=4))
            lino = ctx.enter_context(tc.tile_pool(name="lino", bufs=3))

            # ---- constants into SBUF
            wih_sb = consts.tile([128, KE * 4 * H], b16)
            nc.sync.dma_start(wih_sb[:], wihT.ap())
            whh_sb = consts.tile([128, KH * 4 * H], b16)
            nc.sync.dma_start(whh_sb[:], whhT.ap())
            xs_sb = consts.tile([128, KE * pack], b16)
            nc.sync.dma_start(xs_sb[:], xsT.ap())
            bias_sb = consts.tile([128, GT], f32)
            nc.sync.dma_start(bias_sb[:], biasv.ap())

            gx_sb = gxp.tile([128, GT * pack], b16)
            hs_sb = hsp.tile([128, KH * pack], b16)

            def emit_iteration(_loop):
                # ---- phase 1: Gx for all packed columns
                with tc.tile_pool(name=f"gxps{_loop}", bufs=4, space="PSUM") as gx_ps:
                    for mi in range(GT):
                        for (s, cn) in _col_chunks(pack):
                            ps = gx_ps.tile([128, 512], f32, tag="gx")
                            for k in range(KE):
                                nc.tensor.matmul(
                                    ps[:, :cn],
                                    wih_sb[:, k * 4 * H + mi * 128: k * 4 * H + (mi + 1) * 128],
                                    xs_sb[:, k * pack + s: k * pack + s + cn],
                                    start=(k == 0), stop=(k == KE - 1))
                            nc.scalar.activation(
                                gx_sb[:, mi * pack + s: mi * pack + s + cn],
                                ps[:, :cn],
                                mybir.ActivationFunctionType.Identity,
                                bias=bias_sb[:, mi:mi + 1])

                # ---- phase 2: LSTM steps
                h_prev = hp.tile([128, KH * BS], b16, tag="h")
                nc.sync.dma_start(h_prev[:], h0T.ap())
                c_prev = cp.tile([128, KH * BS], f32, tag="c")
                nc.sync.dma_start(c_prev[:], c0T.ap())

                gate_ctx = tc.tile_pool(name=f"gateps{_loop}", bufs=3, space="PSUM")
                gate_ps = gate_ctx.__enter__()
                for t in range(steps):
                    nt = int(n_sched[t])
                    psA = gate_ps.tile([128, 512], f32, tag="gA")
                    psB = gate_ps.tile([128, 512], f32, tag="gB")
                    for half, ps in ((0, psA), (1, psB)):
                        for j in range(16):
                            mi = half * 16 + j
                            for k in range(KH):
                                nc.tensor.matmul(
                                    ps[:, 32 * j: 32 * j + BS],
                                    whh_sb[:, k * 4 * H + mi * 128: k * 4 * H + (mi + 1) * 128],
                                    h_prev[:, BS * k: BS * (k + 1)],
                                    start=(k == 0), stop=(k == KH - 1))
                    # elementwise
                    o0 = off[t]
                    gxA = gx_sb[:, 0:16 * pack].rearrange(
                        "p (m c) -> p m c", c=pack)[:, :, o0:o0 + nt]
                    gxB = gx_sb[:, 16 * pack:32 * pack].rearrange(
                        "p (m c) -> p m c", c=pack)[:, :, o0:o0 + nt]
                    pA = psA[:, :].rearrange("p (m b) -> p m b", b=BS)[:, :, 0:nt]
                    pB = psB[:, :].rearrange("p (m b) -> p m b", b=BS)[:, :, 0:nt]
                    tA = tmp.tile([128, 512], f32, tag="tA")
                    tB = tmp.tile([128, 512], f32, tag="tB")
                    tAv = tA[:, :].rearrange("p (m b) -> p m b", b=BS)[:, :, 0:nt]
                    tBv = tB[:, :].rearrange("p (m b) -> p m b", b=BS)[:, :, 0:nt]
                    nc.vector.tensor_add(tAv, pA, gxA)
                    nc.vector.tensor_add(tBv, pB, gxB)
                    # bank A = [i(0:256) | g(256:512)], bank B = [f | o]
                    sig_i = tmp.tile([128, 256], f32, tag="sig_i")
                    tgh = tmp.tile([128, 256], f32, tag="tgh")
                    sigB = tmp.tile([128, 512], f32, tag="sigB")
                    nc.scalar.activation(sig_i[:], tA[:, 0:256],
                                         mybir.ActivationFunctionType.Sigmoid)
                    nc.scalar.activation(tgh[:], tA[:, 256:512],
                                         mybir.ActivationFunctionType.Tanh)
                    nc.scalar.activation(sigB[:], tB[:],
                                         mybir.ActivationFunctionType.Sigmoid)
                    v2 = tmp.tile([128, 256], f32, tag="v2")
                    nc.vector.tensor_mul(v2[:], sig_i[:], tgh[:])
                    v1 = tmp.tile([128, 256], f32, tag="v1")
                    nc.vector.tensor_mul(v1[:], sigB[:, 0:256], c_prev[:])
                    c_new = cp.tile([128, KH * BS], f32, tag="c")
                    nc.vector.tensor_add(c_new[:], v1[:], v2[:])
                    tch = tmp.tile([128, 256], f32, tag="tch")
                    nc.scalar.activation(tch[:], c_new[:],
                                         mybir.ActivationFunctionType.Tanh)
                    h_new = hp.tile([128, KH * BS], b16, tag="h")
                    nc.vector.tensor_mul(h_new[:], sigB[:, 256:512], tch[:])
                    # pack live columns into hs
                    hv = h_new[:, :].rearrange("p (k b) -> p k b", b=BS)[:, :, 0:nt]
                    dv = hs_sb[:, :].rearrange("p (k c) -> p k c", c=pack)[:, :, o0:o0 + nt]
                    nc.scalar.copy(dv, hv)
                    h_prev, c_prev = h_new, c_new
                gate_ctx.__exit__(None, None, None)

                # ---- phase 3: vocab linear over packed columns
                with tc.tile_pool(name=f"linps{_loop}", bufs=4, space="PSUM") as lin_ps:
                    for m in range(VT):
                        wt = linw.tile([128, KH * 128], b16, tag="lw")
                        nc.sync.dma_start(wt[:], linT.ap()[:, m * KH * 128:(m + 1) * KH * 128])
                        ps = lin_ps.tile([128, 512], f32, tag="lp")
                        for (s, cn) in _col_chunks(pack):
                            for k in range(KH):
                                nc.tensor.matmul(
                                    ps[:, s:s + cn],
                                    wt[:, k * 128:(k + 1) * 128],
                                    hs_sb[:, k * pack + s: k * pack + s + cn],
                                    start=(k == 0), stop=(k == KH - 1))
                        ob = lino.tile([128, pack], f32, tag="lo")
                        if m % 2 == 0:
                            nc.vector.tensor_copy(ob[:], ps[:, 0:pack])
                        else:
                            nc.scalar.copy(ob[:], ps[:, 0:pack])
                        nc.sync.dma_start(outT.ap()[m * 128:(m + 1) * 128, :], ob[:])

            if dyn_loop:
                nrep_sb = consts.tile([1, 1], mybir.dt.int32)
                nc.sync.dma_start(nrep_sb[:], nrep.ap())
                niter = nc.values_load(nrep_sb[0:1, 0:1], min_val=1, max_val=1 << 20)
                with tc.For_i(0, niter):
                    emit_iteration(0)
            else:
                for _loop in range(loop_n):
                    emit_iteration(_loop)

    return nc


# ---------------------------------------------------------------------------
# Host wrapper
# ---------------------------------------------------------------------------

_compiled = {}


def prepare(features, captions, h0, c0, maxlen, lengths,
            embed_W, W_ih, W_hh, b_ih, b_hh, lin_W, lin_b):
    """Host-side prep: returns (in_maps, meta) where meta carries the packing
    schedule and unpack info."""
    features = np.asarray(features, np.float32)
    captions = np.asarray(captions)
    h0 = np.asarray(h0, np.float32)
    c0 = np.asarray(c0, np.float32)
    lengths = np.asarray(lengths).astype(np.int64)
    embed_W = np.asarray(embed_W, np.float32)
    W_ih = np.asarray(W_ih, np.float32)
    W_hh = np.asarray(W_hh, np.float32)
    b_ih = np.asarray(b_ih, np.float32)
    b_hh = np.asarray(b_hh, np.float32)
    lin_W = np.asarray(lin_W, np.float32)
    lin_b = np.asarray(lin_b, np.float32)
    ml = int(maxlen)
    assert ml == T and features.shape == (B, E)

    # per-core batch assignment, each core's batches sorted by length desc
    cores_bs = []
    for c in range(NC):
        bs = np.array(range(c, B, NC))
        bs = bs[np.argsort(-lengths[bs], kind='stable')]
        cores_bs.append(bs)

    # uniform per-step column counts (max over cores), n_t>0 steps only
    n_ct = np.zeros((NC, T), int)
    for c in range(NC):
        for t in range(T):
            n_ct[c, t] = int((lengths[cores_bs[c]] > t).sum())
    n_sched = n_ct.max(axis=0)
    steps = int((n_sched > 0).sum())
    n_sched = n_sched[:steps]
    assert (np.diff(n_sched) <= 0).all()
    pack = int(n_sched.sum())
    off = np.concatenate([[0], np.cumsum(n_sched)]).astype(int)

    # ---- shared weight layouts
    bias = b_ih + b_hh
    rowb = np.array([_rowbase(mi) for mi in range(GT)])
    # wihT[p, k*4H + mi*128 + q] = W_ih[rowb(mi)+q, 128k+p]
    wih_t = np.zeros((128, KE * 4 * H), np.float32)
    whh_t = np.zeros((128, KH * 4 * H), np.float32)
    for mi in range(GT):
        blk_ih = W_ih[rowb[mi]:rowb[mi] + 128, :]   # [128, E]
        blk_hh = W_hh[rowb[mi]:rowb[mi] + 128, :]   # [128, H]
        for k in range(KE):
            wih_t[:, k * 4 * H + mi * 128:k * 4 * H + (mi + 1) * 128] = \
                blk_ih[:, 128 * k:128 * (k + 1)].T
        for k in range(KH):
            whh_t[:, k * 4 * H + mi * 128:k * 4 * H + (mi + 1) * 128] = \
                blk_hh[:, 128 * k:128 * (k + 1)].T
    biasv = np.zeros((128, GT), np.float32)
    for mi in range(GT):
        biasv[:, mi] = bias[rowb[mi]:rowb[mi] + 128]
    # linT[p, m*KH*128 + k*128 + q] = lin_W[128m+q, 128k+p]
    linp = np.zeros((VP, H), np.float32)
    linp[:V] = lin_W
    lin_t = (linp.reshape(VT, 128, KH, 128)      # [m, q, k, p]
             .transpose(3, 0, 2, 1)              # [p, m, k, q]
             .reshape(128, VT * KH * 128))
    wih_b = wih_t.astype(bf16)
    whh_b = whh_t.astype(bf16)
    lin_b16 = lin_t.astype(bf16)

    # ---- per-core inputs
    in_maps = []
    for c in range(NC):
        bs = cores_bs[c]
        # xs[j, t] = features[bs[j]] (t=0) else embed(captions[bs[j], t-1])
        xs = np.empty((BS, T, E), np.float32)
        xs[:, 0, :] = features[bs]
        xs[:, 1:, :] = embed_W[captions[bs, :T - 1]]
        # packed columns: for t, slots j < n_sched[t]
        xsp = np.zeros((128, KE * pack), np.float32)
        for t in range(steps):
            ntt = int(n_sched[t])
            blk = xs[:ntt, t, :]                  # [ntt, E]
            for k in range(KE):
                xsp[:, k * pack + off[t]: k * pack + off[t] + ntt] = \
                    blk[:, 128 * k:128 * (k + 1)].T
        h0T = np.zeros((128, KH * BS), np.float32)
        c0T = np.zeros((128, KH * BS), np.float32)
        hc = h0[0][bs]   # [BS, H]
        cc = c0[0][bs]
        for k in range(KH):
            h0T[:, BS * k:BS * (k + 1)] = hc[:, 128 * k:128 * (k + 1)].T
            c0T[:, BS * k:BS * (k + 1)] = cc[:, 128 * k:128 * (k + 1)].T
        in_maps.append(dict(
            xsT=xsp.astype(bf16), wihT=wih_b, whhT=whh_b, linT=lin_b16,
            biasv=biasv, h0T=h0T.astype(bf16), c0T=c0T.astype(np.float32)))

    meta = dict(n_sched=n_sched, steps=steps, pack=pack, off=off,
                n_ct=n_ct, cores_bs=cores_bs, lengths=lengths, lin_b=lin_b)
    return in_maps, meta


def unpack(results, meta):
    """results: list of per-core {name: np.ndarray}; -> packed [N, V] fp32."""
    lengths = meta['lengths']
    off = meta['off']
    n_ct = meta['n_ct']
    cores_bs = meta['cores_bs']
    steps = meta['steps']
    Lb, Tb = [], []
    for t in range(T):
        bsel = np.nonzero(lengths > t)[0]
        Lb.append(bsel)
        Tb.append(np.full(bsel.shape, t))
    b_idx = np.concatenate(Lb)
    t_idx = np.concatenate(Tb)
    rowmap = {(int(bb), int(tt)): r
              for r, (bb, tt) in enumerate(zip(b_idx, t_idx))}
    out = np.zeros((len(b_idx), V), np.float32)
    for c in range(NC):
        oT = results[c]["outT"]  # [VP, pack]
        bs = cores_bs[c]
        for t in range(steps):
            ncols = int(n_ct[c, t])
            if ncols == 0:
                continue
            rows = [rowmap[(int(bs[j]), t)] for j in range(ncols)]
            out[rows, :] = oT[:V, off[t]:off[t] + ncols].T
    out += meta['lin_b'][None, :]
    return out


def kernel(**inputs):
    in_maps, meta = prepare(**inputs)
    key = tuple(meta['n_sched'].tolist())
    if key not in _compiled:
        _compiled[key] = build_program(meta['n_sched'], meta['pack'], loop_n=1)
    nc = _compiled[key]
    res = run_bass_kernel_spmd(nc, in_maps, core_ids=list(range(NC)))
    return unpack(res.results, meta)
